# revision 16
# baseline (speedup 1.0000x reference)
"""BitNetLinear forward on 8 TRN2 NeuronCores.

out = x @ (alpha * clip(round(W/alpha), -1, 1))^T
  x [4, 2048, 4096] f32, W [4096, 4096] f32, alpha scalar f32.

Sharding: 2D (4 m-blocks x 2 n-blocks). Core c owns x rows
[2048*(c%4) : +2048] and W rows (out cols) [2048*(c//4) : +2048].
No collectives; host only reshapes/slices (layout) and concatenates.
All arithmetic (ternary quantization + matmul + alpha scaling) on device.

Device kernel (per core):
  - Ternarize via T' = Sign(w + a/2) + Sign(w - a/2) in {-2,0,2} (exact in
    fp8e4/bf16), eviction scale a/2 folds the doubling + alpha.
  - k-split mixed precision: first KF8 of the contraction in fp8e4 using
    DoubleRow matmuls (256 k per MM, ~2x bf16 throughput), rest in bf16.
    x_fp8 quantization noise adds ~2.65e-2 * sqrt(KF8/4096) rel err
    (KF8=1024 -> ~1.34e-2 measured vs the f32 reference; gate is 2e-2).
  - MM schedule (HW-probed): per psum tile, all K accumulated by
    consecutive MMs into the SAME psum bank, 2-bank rotation, ACT evicts.
    This paces ~200 ns/MM steady-state vs ~274 for interleaved-bank loops.
  - W streamed per 512-col n-panel (f32) and quantized on the fly
    (double-buffered); x resident in SBUF as fp8 pairs + bf16 tiles.
"""

import contextlib
import sys

if "/opt/trn_rl_repo" not in sys.path:
    sys.path.insert(0, "/opt/trn_rl_repo")

import numpy as np

import concourse.bass as bass  # noqa: F401
import concourse.mybir as mybir
import concourse.tile as tile
from concourse import bacc
from concourse.bass_utils import run_bass_kernel_spmd

P = 128
N_CORES = 8
D_IN = 4096
D_OUT = 4096
M_TOT = 4 * 2048
MB, NB = 4, 2  # m-blocks x n-blocks
M_SH = M_TOT // MB  # 2048 rows per core
N_SH = D_OUT // NB  # 2048 out-cols per core
KO = D_IN // P  # 32 k-subtiles
N_TILE = 512
MT = M_SH // P  # 16 m-tiles
NP = N_SH // N_TILE  # 4 n-panels

F32 = mybir.dt.float32
BF16 = mybir.dt.bfloat16
FP8 = mybir.dt.float8e4
DR = mybir.MatmulPerfMode.DoubleRow

KF8 = 1536  # leading contraction span done in fp8 DoubleRow


def build(reps=1, kf8=KF8, mode="full"):
    """mode: 'full' (real kernel), 'pe' (probe: memset inputs, no DMA in)."""
    k8 = kf8 // P          # fp8 k-subtiles
    kp8 = k8 // 2          # DoubleRow pairs
    kb = KO - k8           # bf16 k-subtiles
    assert k8 % 2 == 0 and 0 <= k8 <= KO

    nc = bacc.Bacc("TRN2", target_bir_lowering=False, debug=False,
                   num_devices=N_CORES)
    xt_d = nc.declare_dram_parameter("xt", [P, MT, KO, P], F32,
                                     isOutput=False)
    wt_d = nc.declare_dram_parameter("wt", [P, KO, N_SH], F32, isOutput=False)
    al_d = nc.declare_dram_parameter("alpha", [1, 1], F32, isOutput=False)
    out_d = nc.declare_dram_parameter("out", [P, MT, N_SH], F32, isOutput=True)

    with tile.TileContext(nc) as tc:
        with (
            tc.tile_pool(name="const", bufs=1) as const,
            tc.tile_pool(name="xres", bufs=1) as xres_pool,
            tc.tile_pool(name="xstage", bufs=2) as xstage,
            tc.tile_pool(name="wstage", bufs=5) as wstage,
            tc.tile_pool(name="wq", bufs=2) as wqp,
            tc.tile_pool(name="s2", bufs=4) as s2p,
            tc.tile_pool(name="outs", bufs=3) as outs,
            tc.tile_pool(name="psum", bufs=4, space="PSUM") as psum,
        ):
            # alpha -> [128,1]; +a/2 and -a/2 (sign biases, evict scale).
            # Outside the rep loop: rep-invariant, and keeping them inside
            # would serialize reps through WAR on the const tiles (every
            # sign/evict reads them).
            a1 = const.tile([1, 1], F32)
            nc.sync.dma_start(out=a1[:, :], in_=al_d.ap()[:, :])
            ab = const.tile([P, 1], F32)
            nc.gpsimd.partition_broadcast(ab[:, :], a1[:, :])
            half = const.tile([P, 1], F32)
            nc.vector.tensor_scalar_mul(half[:, :], ab[:, :], 0.5)
            neghalf = const.tile([P, 1], F32)
            nc.vector.tensor_scalar_mul(neghalf[:, :], ab[:, :], -0.5)

            rep_ctx = (
                tc.For_i(0, reps, 1) if reps > 1 else contextlib.nullcontext()
            )
            with rep_ctx:
                # resident quantized x: fp8 pairs (k<kf8) + bf16 (k>=kf8)
                x8 = [xres_pool.tile([P, max(k8, 1), P], FP8, name=f"x8_{m}")
                      for m in range(MT)] if k8 else None
                xb = [xres_pool.tile([P, max(kb, 1), P], BF16, name=f"xb_{m}")
                      for m in range(MT)] if kb else None

                def quant_x_mt(m):
                    # stage x[mt] in two contiguous f32 half-DMAs (8KB per
                    # partition each; mt-major host layout), then cast the
                    # k<kf8 span to fp8 and the rest to bf16.
                    H = KO // 2
                    for h in range(2):
                        st = xstage.tile([P, H, P], F32, tag="xst")
                        if mode in ("full", "dma", "now", "noout"):
                            nc.sync.dma_start(
                                out=st[:, :, :],
                                in_=xt_d.ap()[:, m, h * H:(h + 1) * H, :],
                            )
                        else:
                            nc.vector.memset(st[:, :, :], 0.25)
                        if mode == "dma":
                            continue
                        lo, hi = h * H, (h + 1) * H
                        if lo < k8:
                            cut = min(k8, hi)
                            nc.vector.tensor_copy(
                                x8[m][:, lo:cut, :], st[:, 0:cut - lo, :])
                        if hi > k8:
                            cut = max(k8, lo)
                            nc.vector.tensor_copy(
                                xb[m][:, cut - k8:hi - k8, :],
                                st[:, cut - lo:H, :])

                def alloc_w_panel():
                    wq8 = (wqp.tile([P, max(k8, 1), N_TILE], FP8, tag="wq8",
                                    name="wq8") if k8 else None)
                    wqb = (wqp.tile([P, max(kb, 1), N_TILE], BF16, tag="wqb",
                                    name="wqb") if kb else None)
                    return wq8, wqb

                def make_w_chunk(np_, c, wq8, wqb):
                    # stream + ternarize one [2-subtile x 512] chunk of a W
                    # panel: {-2,0,2} as fp8 (k<kf8) / bf16 (k>=kf8).
                    # Emitted chunk-at-a-time inside the m-loop so the ACT
                    # sign ops interleave with psum evicts (ACT is strict
                    # FIFO: a block of 32 signs ahead of an evict stalls
                    # the PE on psum WAR).
                    st = wstage.tile([P, 2, N_TILE], F32, tag="wst")
                    if mode in ("full", "dma", "nox", "noout"):
                        nc.sync.dma_start(
                            out=st[:, :, :],
                            in_=wt_d.ap()[:, 2 * c:2 * c + 2,
                                          np_ * N_TILE:(np_ + 1) * N_TILE],
                        )
                    else:
                        nc.vector.memset(st[:, :, :], 0.5)
                    if mode == "dma":
                        return
                    lo = 2 * c
                    if lo < k8:
                        tgt = wq8[:, lo:lo + 2, :]
                    else:
                        tgt = wqb[:, lo - k8:lo - k8 + 2, :]
                    s2 = s2p.tile([P, 2, N_TILE], BF16, tag="s2")
                    nc.scalar.sign(tgt, st[:, :, :], bias=half[:, :])
                    nc.scalar.sign(s2[:, :, :], st[:, :, :],
                                   bias=neghalf[:, :])
                    nc.vector.tensor_tensor(tgt, tgt, s2[:, :, :],
                                            mybir.AluOpType.add)

                # body top: interleave x quant with panel-0 W production so
                # the DMA ring carries both streams from the start (and, in
                # the rep loop, both overlap the previous rep's tail).
                wq_next = {0: alloc_w_panel()}
                for m in range(MT):
                    quant_x_mt(m)
                    make_w_chunk(0, m, *wq_next[0])

                for np_ in range(NP):
                    wq8, wqb = wq_next.pop(np_)
                    nxt = alloc_w_panel() if np_ + 1 < NP else None
                    for m in range(MT):
                        if mode == "dma":
                            if nxt is not None:
                                make_w_chunk(np_ + 1, m, *nxt)
                            continue
                        ps = psum.tile([P, N_TILE], F32, tag="ps", name="ps")
                        for j in range(kp8):
                            nc.tensor.matmul(
                                ps[:, :],
                                lhsT=x8[m][:, 2 * j:2 * j + 2, :],
                                rhs=wq8[:, 2 * j:2 * j + 2, :],
                                start=(j == 0), stop=False,
                                perf_mode=DR,
                            )
                        for k in range(kb):
                            nc.tensor.matmul(
                                ps[:, :],
                                lhsT=xb[m][:, k, :],
                                rhs=wqb[:, k, :],
                                start=(k == 0 and kp8 == 0),
                                stop=(k == kb - 1),
                            )
                        if kb == 0:
                            # close the group (all-fp8 build): redo last MM
                            # flagging handled above; nothing to do
                            pass
                        ot = outs.tile([P, N_TILE], F32, tag="ot", name="ot")
                        # out = psum * (alpha/2)
                        nc.scalar.mul(ot[:, :], ps[:, :], half[:, :])
                        if mode != "noout":
                            nc.scalar.dma_start(
                                out=out_d.ap()[:, m,
                                               np_ * N_TILE:(np_ + 1) * N_TILE],
                                in_=ot[:, :],
                            )
                        if nxt is not None and m < MT // 2:
                            # next panel's chunks, two per m-tile in the
                            # first half of this panel: the DMA->sign->add
                            # chain finishes well before the next panel
                            # needs it (2-per-mt also keeps ACT interleaved
                            # with evicts).
                            make_w_chunk(np_ + 1, 2 * m, *nxt)
                            make_w_chunk(np_ + 1, 2 * m + 1, *nxt)
                    if nxt is not None:
                        wq_next[np_ + 1] = nxt

    nc.compile()
    return nc


_NC_CACHE = {}


def _get_nc():
    if "nc" not in _NC_CACHE:
        _NC_CACHE["nc"] = build()
    return _NC_CACHE["nc"]


def make_in_maps(x, W, alpha):
    x = np.ascontiguousarray(np.asarray(x, np.float32)).reshape(M_TOT, D_IN)
    W = np.ascontiguousarray(np.asarray(W, np.float32))
    a = np.full((1, 1), np.float32(np.asarray(alpha)), np.float32)
    in_maps = []
    for c in range(N_CORES):
        mi, nj = c % MB, c // MB
        xs = x[mi * M_SH:(mi + 1) * M_SH]
        # xt[p, mt, k, m_in_tile] = xs[mt*128 + m_in_tile, k*128 + p]
        xt = np.ascontiguousarray(
            xs.reshape(MT, P, KO, P).transpose(3, 0, 2, 1))
        ws = W[nj * N_SH:(nj + 1) * N_SH]
        # wt[p, k, n] = ws[n, k*128 + p]
        wt = np.ascontiguousarray(ws.reshape(N_SH, KO, P).transpose(2, 1, 0))
        in_maps.append({"xt": xt, "wt": wt, "alpha": a})
    return in_maps


def gather_out(results):
    full = np.empty((M_TOT, D_OUT), np.float32)
    for c in range(N_CORES):
        mi, nj = c % MB, c // MB
        o = results[c]["out"]  # [P, MT, N_SH]; row = mt*128 + p
        full[mi * M_SH:(mi + 1) * M_SH, nj * N_SH:(nj + 1) * N_SH] = (
            o.transpose(1, 0, 2).reshape(M_SH, N_SH)
        )
    return full.reshape(4, 2048, D_OUT)


def kernel(x, W, alpha):
    nc = _get_nc()
    in_maps = make_in_maps(x, W, alpha)
    res = run_bass_kernel_spmd(nc, in_maps, core_ids=list(range(N_CORES)))
    return gather_out(res.results)
